# revision 8
# baseline (speedup 1.0000x reference)
"""Trainium2 Bass kernel for LocalWindowAttention (v2).

Model: B=2, S=4096, D=1024, H=16 heads, hd=64, window 16 (8 left, 7
right), four dim->dim projections (torch-Linear y = x @ W.T), per-token
windowed softmax.

Sharding: 8 cores = 2 batches x 4 sequence chunks of 1024 tokens, each
with a zero-padded halo (8 left / 7 right, 1039 tokens) so K/V at chunk
boundaries are local - no collectives.

Design (driven by the TRN2 instruction-cost model: a matmul costs
out_free_size cycles; stationary loads are free; PE p-state needs
continuous busy):
  - q/k projections in transposed [dout, tok] layout; v in natural
    [tok, dout] layout, computed in 9 ALIGNED 128-token tiles and then
    re-partitioned into 10 SHINGLED key tiles (tile b = tokens
    [113b, 113b+128)) by partition-shifted SBUF->SBUF DMAs, so each
    attention block's AV contraction is a single matmul.
  - attention blocks of QB=113 q tokens: keys span 113+15 = 128.
  - scoresT layout [keys, q] computed directly (stationary kT, moving
    qT, free = q): exp output (SBUF) feeds AV as stationary with no
    probs transpose and no PSUM->SBUF copy.
  - band mask applied as a 0/1 MULTIPLY on the exp output (exp of raw
    scores is fp16-safe), cheap on DVE.
  - denominators via 16 free=1 matmuls (ones moving operand) into a
    [113 q, 16 h] f32 region bitcast into the tail of the transpose
    PSUM tile (saves a PSUM bank): partition-parallel subtract of the
    edge-pad count ("adj") and reciprocal. Padding keys contribute
    exp(0)=1 to the denominator (k=0) and nothing to the numerator
    (v=0), so edge masking is the exact adj subtraction.
  - AV: out[q, hd] natural (stationary expT, moving v, free = hd);
    normalization fused into the PSUM->SBUF copy (multiply by the
    per-partition rinv).
  - attnT via PE transposes (slot stride 114 keeps fp16 PSUM writes
    4-byte aligned); the out projection consumes attnT as stationary.
    Head slots are grouped by PE parity (PSUM tile_position rule), so
    Wo rows are permuted on the host to match the slot order.
  - out-proj results copied PSUM->SBUF fp16 on the Scalar engine and
    DMA'd as fp16; host upcasts.
  - schedule: big-bite DMAs (HWDGE descriptor gen is ~630ns serial per
    DMA) with quarter-bites for the startup-critical wq/xT; q/k
    projections first; v-proj units, k-proj tail and out-proj chunks
    are PE "filler" interleaved into a software-pipelined attention
    loop (block b's AV/normalize/transpose stages overlap block b+1's
    scores/exp) so the PE almost never idles.
"""

import numpy as np

import concourse.bass as bass
import concourse.mybir as mybir
import concourse.tile as tile
from concourse import bacc
from concourse.bass_utils import run_bass_kernel_spmd
from concourse.masks import make_identity

F16 = mybir.dt.float16
F32 = mybir.dt.float32

B, S, D = 2, 4096, 1024
H, HD = 16, 64
WIN, LP, RP = 16, 8, 7
NCORES = 8
CHUNK = S // 4            # tokens per core
TH = CHUNK + LP + RP      # halo token count (1039)
DT = D // 128             # 128-row tiles across D (8)
QB = 113                  # q tokens per attention block (keys = QB+15 = 128)
NBLK = (CHUNK + QB - 1) // QB   # 10 blocks (9 full + 7-token tail)
QTAIL = CHUNK - QB * (NBLK - 1)  # 7
NOUT = CHUNK // 128       # out-proj token tiles (8)

# head slots grouped by PE parity (stationary base partition must be
# uniform within a PSUM bank): slots 0-7 = even heads, 8-15 = odd.
SLOT2HEAD = [0, 2, 4, 6, 8, 10, 12, 14, 1, 3, 5, 7, 9, 11, 13, 15]

TRACE = False             # test.py may set kernel.TRACE = True
LAST_RESULTS = None       # BassKernelResults of the most recent run

_PROGRAM = None


def _blk(b):
    """(t0, qn, kn) for block b: key origin, q count, key count."""
    qn = QB if b < NBLK - 1 else QTAIL
    kn = qn + WIN - 1
    return QB * b, qn, kn


def _build_program():
    nc = bacc.Bacc("TRN2", target_bir_lowering=False, debug=False)

    xT_d = nc.dram_tensor("xT", [D, TH], F16, kind="ExternalInput")
    wq_d = nc.dram_tensor("wqT", [D, D], F16, kind="ExternalInput")
    wk_d = nc.dram_tensor("wkT", [D, D], F16, kind="ExternalInput")
    wv_d = nc.dram_tensor("wvT", [D, D], F16, kind="ExternalInput")
    wo_d = nc.dram_tensor("woT", [D, D], F16, kind="ExternalInput")
    adj_d = nc.dram_tensor("adj", [QB, NBLK], F32, kind="ExternalInput")
    band_d = nc.dram_tensor("bandT", [128, QB], F16, kind="ExternalInput")
    ones_d = nc.dram_tensor("ones", [128, 1], F16, kind="ExternalInput")
    out_d = nc.dram_tensor("out", [CHUNK, D], F16, kind="ExternalOutput")

    with tile.TileContext(nc) as tc:
        with (
            tc.tile_pool(name="const", bufs=1) as cpool,
            tc.tile_pool(name="acts", bufs=1) as apool,
            tc.tile_pool(name="soft", bufs=2) as spool,
            tc.tile_pool(name="outsb", bufs=3) as opool,
        ):
            # ---- constants / activations resident in SBUF ----
            identity = cpool.tile([128, 128], F16)
            make_identity(nc, identity)

            # Weights live whole in SBUF: [128, k, dout]. One or two DMAs
            # each — descriptor generation (HWDGE) is serial and ~630ns
            # per DMA, so few big DMAs beat many small ones.
            xT = apool.tile([128, DT, TH], F16)
            wq_sb = apool.tile([128, DT, D], F16)
            wk_sb = apool.tile([128, DT, D], F16)
            wv_sb = apool.tile([128, DT, D], F16)
            wo_sb = apool.tile([128, DT, D], F16)
            xsrc = xT_d.ap().rearrange("(j p) t -> p j t", p=128)
            # startup-critical: stream wq m-columns and xT tokens in
            # quarter bites so the first q-proj matmuls start ~3us in
            wqsrc = wq_d.ap().rearrange("(j p) o -> p j o", p=128)
            nc.sync.dma_start(wq_sb[:, :, 0:256], wqsrc[:, :, 0:256])
            nc.sync.dma_start(xT[:, :, 0:264], xsrc[:, :, 0:264])
            nc.sync.dma_start(wq_sb[:, :, 256:512], wqsrc[:, :, 256:512])
            nc.sync.dma_start(xT[:, :, 264:520], xsrc[:, :, 264:520])
            nc.sync.dma_start(wq_sb[:, :, 512:D], wqsrc[:, :, 512:D])
            nc.sync.dma_start(xT[:, :, 520:TH], xsrc[:, :, 520:TH])
            nc.sync.dma_start(
                wk_sb, wk_d.ap().rearrange("(j p) o -> p j o", p=128)
            )

            qT = apool.tile([128, DT, CHUNK], F16)
            kT = apool.tile([128, DT, TH], F16)
            v_al = apool.tile([128, 9, H, HD], F16)
            v_sb = apool.tile([128, NBLK, H, HD], F16)
            attnT = apool.tile([128, DT, CHUNK], F16)

            # ---- phase A: q/k projections [dout_tile, tokens] ----
            with (
                tc.tile_pool(name="fill_ps", bufs=2, space="PSUM") as fill_ps,
                tc.tile_pool(name="sc_ps", bufs=2, space="PSUM") as sc_ps,
                tc.tile_pool(name="av_ps", bufs=2, space="PSUM") as av_ps,
                tc.tile_pool(name="tr_ps", bufs=2, space="PSUM") as tr_ps,
            ):
                def qk_chunk(w_sb, dst, m, src_c0, cn, dst_c0):
                    ps = fill_ps.tile([128, 512], F32, tag="fill")
                    for k in range(DT):
                        nc.tensor.matmul(
                            ps[:, :cn],
                            w_sb[:, k, m * 128:(m + 1) * 128],
                            xT[:, k, src_c0: src_c0 + cn],
                            start=(k == 0),
                            stop=(k == DT - 1),
                        )
                    nc.vector.tensor_copy(dst[:, m, dst_c0:dst_c0 + cn], ps[:, :cn])

                # q-proj: tokens [LP, LP+1024); follow the quarter-bite
                # DMA stream: m 0-1 on 256-token chunks first
                for m in range(2):
                    qk_chunk(wq_sb, qT, m, LP, 256, 0)
                for m in range(2):
                    qk_chunk(wq_sb, qT, m, LP + 256, 256, 256)
                for m in range(2, 4):
                    qk_chunk(wq_sb, qT, m, LP, 512, 0)
                ones_sb = cpool.tile([128, 1], F16)
                band_sb = cpool.tile([128, QB], F16)
                adj_sb = cpool.tile([QB, NBLK], F32)
                nc.sync.dma_start(ones_sb, ones_d.ap())
                nc.sync.dma_start(band_sb, band_d.ap())
                nc.sync.dma_start(adj_sb, adj_d.ap())
                nc.sync.dma_start(
                    wv_sb, wv_d.ap().rearrange("(j p) o -> p j o", p=128)
                )
                for m in range(4, DT):
                    qk_chunk(wq_sb, qT, m, LP, 512, 0)
                for m in range(DT):
                    qk_chunk(wq_sb, qT, m, LP + 512, 512, 512)

                # k-proj main chunks over [0, 1024)
                for m in range(DT):
                    qk_chunk(wk_sb, kT, m, 0, 512, 0)
                nc.sync.dma_start(
                    wo_sb, wo_d.ap().rearrange("(j p) o -> p j o", p=128)
                )
                for m in range(DT):
                    qk_chunk(wk_sb, kT, m, 512, 512, 512)

                # k-proj tail [1024, 1039): all 8 m-tiles batched into one
                # PSUM tile (8 accumulation groups, 16-col aligned slots),
                # one copy out
                kt = fill_ps.tile([128, DT, 16], F32, tag="fill")
                for m in range(DT):
                    for k in range(DT):
                        nc.tensor.matmul(
                            kt[:, m, 0:TH - 1024],
                            wk_sb[:, k, m * 128:(m + 1) * 128],
                            xT[:, k, 1024:TH],
                            start=(k == 0),
                            stop=(k == DT - 1),
                        )
                nc.vector.tensor_copy(
                    kT[:, :, 1024:TH], kt[:, :, 0:TH - 1024]
                )

                # ---- phase B: v-proj + attention + out-proj ----
                # filler units: one PSUM-group of 8 matmuls each.
                # aligned-v units first (with shingle DMAs draining as
                # sources complete), then out-proj chunks.
                vq = [(a, n) for a in range(9) for n in range(2)]
                shq = list(range(NBLK))
                opj = [(t, n) for t in range(NOUT) for n in range(2)]
                done_blocks = 0
                v_done = -1

                def shingle_src_max(b):
                    t0, qn, kn = _blk(b)
                    a, off = divmod(QB * b, 128)
                    return a if kn <= 128 - off else a + 1

                def emit_v(a, n):
                    # aligned v tile a = tokens [128a, min(128a+128, TH))
                    rows = min(128, TH - 128 * a)
                    ps = fill_ps.tile([128, 512], F32, tag="fill")
                    for k in range(DT):
                        nc.tensor.matmul(
                            ps[:rows, :],
                            xT[:, k, 128 * a: 128 * a + rows],
                            wv_sb[:, k, n * 512:(n + 1) * 512],
                            start=(k == 0),
                            stop=(k == DT - 1),
                        )
                    nc.vector.tensor_copy(
                        v_al[:rows, a, 8 * n: 8 * n + 8, :], ps[:rows, :]
                    )

                def emit_shingle(b):
                    # shingled key tile b = tokens [113b, 113b+kn) copied
                    # out of the aligned tiles (partition-shifted SBUF DMA)
                    t0, qn, kn = _blk(b)
                    a, off = divmod(QB * b, 128)
                    len1 = min(128 - off, kn)
                    nc.sync.dma_start(
                        v_sb[0:len1, b], v_al[off: off + len1, a]
                    )
                    if kn > len1:
                        nc.sync.dma_start(
                            v_sb[len1:kn, b], v_al[0: kn - len1, a + 1]
                        )

                def emit_opj(t, n, cols=512):
                    for c0 in range(n * 512, (n + 1) * 512, cols):
                        ps = fill_ps.tile([128, 512], F32, tag="fill")
                        for k in range(DT):
                            nc.tensor.matmul(
                                ps[:, :cols],
                                attnT[:, k, t * 128:(t + 1) * 128],
                                wo_sb[:, k, c0: c0 + cols],
                                start=(k == 0),
                                stop=(k == DT - 1),
                            )
                        osb = opool.tile([128, 512], F16, tag="osb")
                        nc.scalar.copy(osb[:, :cols], ps[:, :cols])
                        nc.sync.dma_start(
                            out_d.ap()[t * 128:(t + 1) * 128, c0: c0 + cols],
                            osb[:, :cols],
                        )

                def pop_v():
                    nonlocal v_done
                    a, n = vq.pop(0)
                    emit_v(a, n)
                    if n == 1:
                        v_done = a
                    while shq and shingle_src_max(shq[0]) <= v_done:
                        emit_shingle(shq.pop(0))

                def filler(budget, sh_until=-1):
                    """Emit filler PE work: aligned-v units until blocks
                    <= sh_until have their shingled v, then up to `budget`
                    units total from the v queue and ready out-proj."""
                    emitted = 0
                    while shq and shq[0] <= sh_until:
                        pop_v()
                        emitted += 1
                    while emitted < budget:
                        if vq:
                            pop_v()
                        elif opj:
                            t, n = opj[0]
                            need = -(-(128 * (t + 1)) // QB)  # ceil
                            if done_blocks < need:
                                break
                            opj.pop(0)
                            emit_opj(t, n)
                        else:
                            break
                        emitted += 1

                expT_of = {}

                def emit_scores(b, gh):
                    """Scores+exp+band for groups (2gh, 2gh+1) of block b."""
                    t0, qn, kn = _blk(b)
                    expT = expT_of[b]
                    for g in (2 * gh, 2 * gh + 1):
                        sc = sc_ps.tile([128, 4, QB], F32, tag="sc")
                        for i in range(4):
                            h = SLOT2HEAD[4 * g + i]
                            l, m = h & 1, h // 2
                            nc.tensor.matmul(
                                sc[:kn, i, :qn],
                                kT[64 * l:64 * l + 64, m, t0: t0 + kn],
                                qT[64 * l:64 * l + 64, m, QB * b: QB * b + qn],
                                start=True,
                                stop=True,
                            )
                        nc.scalar.activation(
                            expT[:kn, 4 * g: 4 * g + 4, :qn],
                            sc[:kn, :, :qn],
                            mybir.ActivationFunctionType.Exp,
                            scale=0.125,
                        )
                        nc.vector.tensor_tensor(
                            expT[:kn, 4 * g: 4 * g + 4, :qn],
                            expT[:kn, 4 * g: 4 * g + 4, :qn],
                            band_sb[:kn, None, :qn].broadcast_to([kn, 4, qn]),
                            mybir.AluOpType.mult,
                        )

                # prologue: shingled v 0/1, then block-0 scores
                filler(0, sh_until=1)
                expT_of[0] = spool.tile([128, H, QB], F16, tag="expT",
                                        name="expT_0")
                emit_scores(0, 0)
                emit_scores(0, 1)

                # software-pipelined main loop: block b's AV/denominator/
                # normalize/transpose stages interleaved with block b+1's
                # scores and with v-proj / out-proj PE filler
                for b in range(NBLK):
                    t0, qn, kn = _blk(b)
                    expT = expT_of[b]
                    rinv = spool.tile([QB, H], F32, tag="rinv")
                    attn_sb = spool.tile([QB, 2, 8, HD], F16, tag="attn")
                    if b + 1 < NBLK:
                        expT_of[b + 1] = spool.tile(
                            [128, H, QB], F16, tag="expT",
                            name=f"expT_{b + 1}"
                        )

                    # transpose tile also hosts the denominators in its
                    # tail bytes (bitcast f32) - saves a PSUM bank
                    tr = tr_ps.tile([128, DT * 114 + 2 * H], F16, tag="tr")
                    dn = tr[0:QB, DT * 114: DT * 114 + 2 * H].bitcast(F32)
                    av_tiles = []
                    for half in range(2):
                        for j in range(8):
                            s = 8 * half + j
                            nc.tensor.matmul(
                                dn[:qn, s:s + 1],
                                expT[:kn, s, :qn],
                                ones_sb[:kn, :],
                                start=True,
                                stop=True,
                            )
                        av = av_ps.tile([QB, 8, HD], F32, tag="av")
                        av_tiles.append(av)
                        for j in range(8):
                            s = 8 * half + j
                            h = SLOT2HEAD[s]
                            nc.tensor.matmul(
                                av[:qn, j, :],
                                expT[:kn, s, :qn],
                                v_sb[:kn, b, h, :],
                                start=True,
                                stop=True,
                            )
                        if b + 1 < NBLK:
                            emit_scores(b + 1, half)
                        else:
                            filler(1)

                    # rinv = 1 / (sum - edge_pad_count)
                    nc.vector.tensor_tensor(
                        rinv[:qn],
                        dn[:qn],
                        adj_sb[:qn, b, None].broadcast_to([qn, H]),
                        mybir.AluOpType.subtract,
                    )
                    nc.vector.reciprocal(rinv[:qn], rinv[:qn])

                    # normalize fused into PSUM->SBUF copy
                    for half in range(2):
                        nc.vector.tensor_tensor(
                            attn_sb[:qn, half],
                            av_tiles[half][:qn],
                            rinv[:qn, 8 * half: 8 * half + 8, None].broadcast_to(
                                [qn, 8, HD]
                            ),
                            mybir.AluOpType.mult,
                        )

                    filler(2, sh_until=min(b + 2, NBLK - 1))

                    # attnT via PE transposes (2-slot feature chunks);
                    # slot stride padded to 114 so each fp16 PSUM write
                    # starts 4-byte aligned (walrus verifier rule)
                    tr = tr_ps.tile([128, DT, QB + 1], F16, tag="tr")
                    for half in range(2):
                        for jj in range(4):
                            nc.tensor.transpose(
                                tr[:, 4 * half + jj, :qn],
                                attn_sb[:qn, half, 2 * jj: 2 * jj + 2, :],
                                identity[:qn, :qn],
                            )
                    nc.vector.tensor_copy(
                        attnT[:, :, QB * b: QB * b + qn], tr[:, :, :qn]
                    )
                    done_blocks = b + 1

                # drain: split the last chunks so the final PSUM->SBUF
                # copy + DMA chain is short
                while opj:
                    t, n = opj.pop(0)
                    emit_opj(t, n, cols=256 if not opj else 512)

    nc.compile()
    return nc


def _get_program():
    global _PROGRAM
    if _PROGRAM is None:
        _PROGRAM = _build_program()
    return _PROGRAM


def _host_inputs(x, Wq, Wk, Wv, Wo):
    """Shard + preprocess full inputs into per-core input maps."""
    x = np.asarray(x, dtype=np.float32)
    wts = {}
    for name, w in (("wqT", Wq), ("wkT", Wk), ("wvT", Wv)):
        wts[name] = np.ascontiguousarray(np.asarray(w, np.float32).T).astype(
            np.float16
        )
    # Wo rows permuted to the head-slot order of attnT
    woT = np.ascontiguousarray(np.asarray(Wo, np.float32).T)
    perm = np.array(
        [64 * SLOT2HEAD[f // 64] + f % 64 for f in range(D)], np.int64
    )
    wts["woT"] = woT[perm].astype(np.float16)

    # band mask in [key, q] layout: valid iff q <= k <= q+15
    kk = np.arange(128)[:, None]
    qq = np.arange(QB)[None, :]
    band = ((kk >= qq) & (kk <= qq + WIN - 1)).astype(np.float16)
    ones = np.ones((128, 1), np.float16)

    in_maps = []
    for c in range(NCORES):
        bb, chunk = divmod(c, 4)
        g0 = chunk * CHUNK
        lo, hi = g0 - LP, g0 + CHUNK + RP
        xpad = np.zeros((TH, D), np.float32)
        src_lo, src_hi = max(lo, 0), min(hi, S)
        xpad[src_lo - lo: src_hi - lo] = x[bb, src_lo:src_hi]
        xT = np.ascontiguousarray(xpad.T).astype(np.float16)

        glob = g0 + np.arange(CHUNK)
        pos = glob[:, None] - LP + np.arange(WIN)[None, :]
        counts = ((pos < 0) | (pos >= S)).sum(axis=1).astype(np.float32)
        adj = np.zeros((QB, NBLK), np.float32)
        for b in range(NBLK):
            qn = QB if b < NBLK - 1 else QTAIL
            adj[:qn, b] = counts[QB * b: QB * b + qn]

        in_maps.append(
            {"xT": xT, "adj": adj, "bandT": band, "ones": ones, **wts}
        )
    return in_maps


def kernel(x, Wq, Wk, Wv, Wo):
    global LAST_RESULTS
    nc = _get_program()
    in_maps = _host_inputs(x, Wq, Wk, Wv, Wo)
    res = run_bass_kernel_spmd(
        nc, in_maps, core_ids=list(range(NCORES)), trace=TRACE
    )
    LAST_RESULTS = res
    out = np.empty((B, S, D), np.float32)
    for c in range(NCORES):
        bb, chunk = divmod(c, 4)
        out[bb, chunk * CHUNK:(chunk + 1) * CHUNK] = (
            res.results[c]["out"].astype(np.float32)
        )
    return out


# revision 9
# speedup vs baseline: 1.0064x; 1.0064x over previous
"""Trainium2 Bass kernel for LocalWindowAttention (v2).

Model: B=2, S=4096, D=1024, H=16 heads, hd=64, window 16 (8 left, 7
right), four dim->dim projections (torch-Linear y = x @ W.T), per-token
windowed softmax.

Sharding: 8 cores = 2 batches x 4 sequence chunks of 1024 tokens, each
with a zero-padded halo (8 left / 7 right, 1039 tokens) so K/V at chunk
boundaries are local - no collectives.

Design (driven by the TRN2 instruction-cost model: a matmul costs
out_free_size cycles; stationary loads are free; PE p-state needs
continuous busy):
  - q/k projections in transposed [dout, tok] layout; v in natural
    [tok, dout] layout, computed in 9 ALIGNED 128-token tiles and then
    re-partitioned into 10 SHINGLED key tiles (tile b = tokens
    [113b, 113b+128)) by partition-shifted SBUF->SBUF DMAs, so each
    attention block's AV contraction is a single matmul.
  - attention blocks of QB=113 q tokens: keys span 113+15 = 128.
  - scoresT layout [keys, q] computed directly (stationary kT, moving
    qT, free = q): exp output (SBUF) feeds AV as stationary with no
    probs transpose and no PSUM->SBUF copy.
  - band mask applied as a 0/1 MULTIPLY on the exp output (exp of raw
    scores is fp16-safe), cheap on DVE.
  - denominators via 16 free=1 matmuls (ones moving operand) into a
    [113 q, 16 h] f32 region bitcast into the tail of the transpose
    PSUM tile (saves a PSUM bank): partition-parallel subtract of the
    edge-pad count ("adj") and reciprocal. Padding keys contribute
    exp(0)=1 to the denominator (k=0) and nothing to the numerator
    (v=0), so edge masking is the exact adj subtraction.
  - AV: out[q, hd] natural (stationary expT, moving v, free = hd);
    normalization fused into the PSUM->SBUF copy (multiply by the
    per-partition rinv).
  - attnT via PE transposes (slot stride 114 keeps fp16 PSUM writes
    4-byte aligned); the out projection consumes attnT as stationary.
    Head slots are grouped by PE parity (PSUM tile_position rule), so
    Wo rows are permuted on the host to match the slot order.
  - out-proj results copied PSUM->SBUF fp16 on the Scalar engine and
    DMA'd as fp16; host upcasts.
  - schedule: big-bite DMAs (HWDGE descriptor gen is ~630ns serial per
    DMA) with quarter-bites for the startup-critical wq/xT; q/k
    projections first; v-proj units, k-proj tail and out-proj chunks
    are PE "filler" interleaved into a software-pipelined attention
    loop (block b's AV/normalize/transpose stages overlap block b+1's
    scores/exp) so the PE almost never idles.
"""

import numpy as np

import concourse.bass as bass
import concourse.mybir as mybir
import concourse.tile as tile
from concourse import bacc
from concourse.bass_utils import run_bass_kernel_spmd
from concourse.masks import make_identity

F16 = mybir.dt.float16
F32 = mybir.dt.float32

B, S, D = 2, 4096, 1024
H, HD = 16, 64
WIN, LP, RP = 16, 8, 7
NCORES = 8
CHUNK = S // 4            # tokens per core
TH = CHUNK + LP + RP      # halo token count (1039)
DT = D // 128             # 128-row tiles across D (8)
QB = 113                  # q tokens per attention block (keys = QB+15 = 128)
NBLK = (CHUNK + QB - 1) // QB   # 10 blocks (9 full + 7-token tail)
QTAIL = CHUNK - QB * (NBLK - 1)  # 7
NOUT = CHUNK // 128       # out-proj token tiles (8)

# head slots grouped by PE parity (stationary base partition must be
# uniform within a PSUM bank): slots 0-7 = even heads, 8-15 = odd.
SLOT2HEAD = [0, 2, 4, 6, 8, 10, 12, 14, 1, 3, 5, 7, 9, 11, 13, 15]

TRACE = False             # test.py may set kernel.TRACE = True
LAST_RESULTS = None       # BassKernelResults of the most recent run

_PROGRAM = None


def _blk(b):
    """(t0, qn, kn) for block b: key origin, q count, key count."""
    qn = QB if b < NBLK - 1 else QTAIL
    kn = qn + WIN - 1
    return QB * b, qn, kn


def _build_program():
    nc = bacc.Bacc("TRN2", target_bir_lowering=False, debug=False)

    xT_d = nc.dram_tensor("xT", [D, TH], F16, kind="ExternalInput")
    wq_d = nc.dram_tensor("wqT", [D, D], F16, kind="ExternalInput")
    wk_d = nc.dram_tensor("wkT", [D, D], F16, kind="ExternalInput")
    wv_d = nc.dram_tensor("wvT", [D, D], F16, kind="ExternalInput")
    wo_d = nc.dram_tensor("woT", [D, D], F16, kind="ExternalInput")
    adj_d = nc.dram_tensor("adj", [QB, NBLK], F32, kind="ExternalInput")
    band_d = nc.dram_tensor("bandT", [128, QB], F16, kind="ExternalInput")
    ones_d = nc.dram_tensor("ones", [128, 1], F16, kind="ExternalInput")
    out_d = nc.dram_tensor("out", [CHUNK, D], F16, kind="ExternalOutput")

    with tile.TileContext(nc) as tc:
        with (
            tc.tile_pool(name="const", bufs=1) as cpool,
            tc.tile_pool(name="acts", bufs=1) as apool,
            tc.tile_pool(name="soft", bufs=2) as spool,
            tc.tile_pool(name="outsb", bufs=3) as opool,
        ):
            # ---- constants / activations resident in SBUF ----
            identity = cpool.tile([128, 128], F16)
            make_identity(nc, identity)

            # Weights live whole in SBUF: [128, k, dout]. One or two DMAs
            # each — descriptor generation (HWDGE) is serial and ~630ns
            # per DMA, so few big DMAs beat many small ones.
            xT = apool.tile([128, DT, TH], F16)
            wq_sb = apool.tile([128, DT, D], F16)
            wk_sb = apool.tile([128, DT, D], F16)
            wv_sb = apool.tile([128, DT, D], F16)
            wo_sb = apool.tile([128, DT, D], F16)
            xsrc = xT_d.ap().rearrange("(j p) t -> p j t", p=128)
            # startup-critical: stream wq m-columns and xT tokens in
            # quarter bites so the first q-proj matmuls start ~3us in
            wqsrc = wq_d.ap().rearrange("(j p) o -> p j o", p=128)
            nc.sync.dma_start(wq_sb[:, :, 0:256], wqsrc[:, :, 0:256])
            nc.sync.dma_start(xT[:, :, 0:264], xsrc[:, :, 0:264])
            nc.sync.dma_start(wq_sb[:, :, 256:512], wqsrc[:, :, 256:512])
            nc.sync.dma_start(xT[:, :, 264:520], xsrc[:, :, 264:520])
            nc.sync.dma_start(wq_sb[:, :, 512:D], wqsrc[:, :, 512:D])
            nc.sync.dma_start(xT[:, :, 520:TH], xsrc[:, :, 520:TH])
            nc.sync.dma_start(
                wk_sb, wk_d.ap().rearrange("(j p) o -> p j o", p=128)
            )

            qT = apool.tile([128, DT, CHUNK], F16)
            kT = apool.tile([128, DT, TH], F16)
            v_al = apool.tile([128, 9, H, HD], F16)
            v_sb = apool.tile([128, NBLK, H, HD], F16)
            attnT = apool.tile([128, DT, CHUNK], F16)

            # ---- phase A: q/k projections [dout_tile, tokens] ----
            with (
                tc.tile_pool(name="fill_ps", bufs=2, space="PSUM") as fill_ps,
                tc.tile_pool(name="sc_ps", bufs=2, space="PSUM") as sc_ps,
                tc.tile_pool(name="av_ps", bufs=2, space="PSUM") as av_ps,
                tc.tile_pool(name="tr_ps", bufs=2, space="PSUM") as tr_ps,
            ):
                def qk_chunk(w_sb, dst, m, src_c0, cn, dst_c0):
                    ps = fill_ps.tile([128, 512], F32, tag="fill")
                    for k in range(DT):
                        nc.tensor.matmul(
                            ps[:, :cn],
                            w_sb[:, k, m * 128:(m + 1) * 128],
                            xT[:, k, src_c0: src_c0 + cn],
                            start=(k == 0),
                            stop=(k == DT - 1),
                        )
                    nc.vector.tensor_copy(dst[:, m, dst_c0:dst_c0 + cn], ps[:, :cn])

                # PE warm-up: the p-state ramp reaches full speed only
                # after 3us of continuous busy, so chew on the identity
                # tile while the first wq/xT DMAs stream. Results land in
                # a scratch PSUM tile nobody reads.
                warm = fill_ps.tile([128, 512], F32, tag="fill")
                for _ in range(44):
                    nc.tensor.matmul(
                        warm[:, 0:128], identity, identity,
                        start=True, stop=True,
                    )

                # q-proj: tokens [LP, LP+1024); follow the quarter-bite
                # DMA stream: m 0-1 on 256-token chunks first
                for m in range(2):
                    qk_chunk(wq_sb, qT, m, LP, 256, 0)
                for m in range(2):
                    qk_chunk(wq_sb, qT, m, LP + 256, 256, 256)
                for m in range(2, 4):
                    qk_chunk(wq_sb, qT, m, LP, 512, 0)
                ones_sb = cpool.tile([128, 1], F16)
                band_sb = cpool.tile([128, QB], F16)
                adj_sb = cpool.tile([QB, NBLK], F32)
                nc.sync.dma_start(ones_sb, ones_d.ap())
                nc.sync.dma_start(band_sb, band_d.ap())
                nc.sync.dma_start(adj_sb, adj_d.ap())
                nc.sync.dma_start(
                    wv_sb, wv_d.ap().rearrange("(j p) o -> p j o", p=128)
                )
                for m in range(4, DT):
                    qk_chunk(wq_sb, qT, m, LP, 512, 0)
                for m in range(DT):
                    qk_chunk(wq_sb, qT, m, LP + 512, 512, 512)

                # k-proj main chunks over [0, 1024)
                for m in range(DT):
                    qk_chunk(wk_sb, kT, m, 0, 512, 0)
                nc.sync.dma_start(
                    wo_sb, wo_d.ap().rearrange("(j p) o -> p j o", p=128)
                )
                for m in range(DT):
                    qk_chunk(wk_sb, kT, m, 512, 512, 512)

                # k-proj tail [1024, 1039): all 8 m-tiles batched into one
                # PSUM tile (8 accumulation groups, 16-col aligned slots),
                # one copy out
                kt = fill_ps.tile([128, DT, 16], F32, tag="fill")
                for m in range(DT):
                    for k in range(DT):
                        nc.tensor.matmul(
                            kt[:, m, 0:TH - 1024],
                            wk_sb[:, k, m * 128:(m + 1) * 128],
                            xT[:, k, 1024:TH],
                            start=(k == 0),
                            stop=(k == DT - 1),
                        )
                nc.vector.tensor_copy(
                    kT[:, :, 1024:TH], kt[:, :, 0:TH - 1024]
                )

                # ---- phase B: v-proj + attention + out-proj ----
                # filler units: one PSUM-group of 8 matmuls each.
                # aligned-v units first (with shingle DMAs draining as
                # sources complete), then out-proj chunks.
                vq = [(a, n) for a in range(9) for n in range(2)]
                shq = list(range(NBLK))
                opj = [(t, n) for t in range(NOUT) for n in range(2)]
                done_blocks = 0
                v_done = -1

                def shingle_src_max(b):
                    t0, qn, kn = _blk(b)
                    a, off = divmod(QB * b, 128)
                    return a if kn <= 128 - off else a + 1

                def emit_v(a, n):
                    # aligned v tile a = tokens [128a, min(128a+128, TH))
                    rows = min(128, TH - 128 * a)
                    ps = fill_ps.tile([128, 512], F32, tag="fill")
                    for k in range(DT):
                        nc.tensor.matmul(
                            ps[:rows, :],
                            xT[:, k, 128 * a: 128 * a + rows],
                            wv_sb[:, k, n * 512:(n + 1) * 512],
                            start=(k == 0),
                            stop=(k == DT - 1),
                        )
                    nc.vector.tensor_copy(
                        v_al[:rows, a, 8 * n: 8 * n + 8, :], ps[:rows, :]
                    )

                def emit_shingle(b):
                    # shingled key tile b = tokens [113b, 113b+kn) copied
                    # out of the aligned tiles (partition-shifted SBUF DMA)
                    t0, qn, kn = _blk(b)
                    a, off = divmod(QB * b, 128)
                    len1 = min(128 - off, kn)
                    nc.sync.dma_start(
                        v_sb[0:len1, b], v_al[off: off + len1, a]
                    )
                    if kn > len1:
                        nc.sync.dma_start(
                            v_sb[len1:kn, b], v_al[0: kn - len1, a + 1]
                        )

                def emit_opj(t, n, cols=512):
                    for c0 in range(n * 512, (n + 1) * 512, cols):
                        ps = fill_ps.tile([128, 512], F32, tag="fill")
                        for k in range(DT):
                            nc.tensor.matmul(
                                ps[:, :cols],
                                attnT[:, k, t * 128:(t + 1) * 128],
                                wo_sb[:, k, c0: c0 + cols],
                                start=(k == 0),
                                stop=(k == DT - 1),
                            )
                        osb = opool.tile([128, 512], F16, tag="osb")
                        nc.scalar.copy(osb[:, :cols], ps[:, :cols])
                        nc.sync.dma_start(
                            out_d.ap()[t * 128:(t + 1) * 128, c0: c0 + cols],
                            osb[:, :cols],
                        )

                def pop_v():
                    nonlocal v_done
                    a, n = vq.pop(0)
                    emit_v(a, n)
                    if n == 1:
                        v_done = a
                    while shq and shingle_src_max(shq[0]) <= v_done:
                        emit_shingle(shq.pop(0))

                def filler(budget, sh_until=-1):
                    """Emit filler PE work: aligned-v units until blocks
                    <= sh_until have their shingled v, then up to `budget`
                    units total from the v queue and ready out-proj."""
                    emitted = 0
                    while shq and shq[0] <= sh_until:
                        pop_v()
                        emitted += 1
                    while emitted < budget:
                        if vq:
                            pop_v()
                        elif opj:
                            t, n = opj[0]
                            need = -(-(128 * (t + 1)) // QB)  # ceil
                            if done_blocks < need:
                                break
                            opj.pop(0)
                            emit_opj(t, n)
                        else:
                            break
                        emitted += 1

                expT_of = {}

                def emit_scores(b, gh):
                    """Scores+exp+band for groups (2gh, 2gh+1) of block b."""
                    t0, qn, kn = _blk(b)
                    expT = expT_of[b]
                    for g in (2 * gh, 2 * gh + 1):
                        sc = sc_ps.tile([128, 4, QB], F32, tag="sc")
                        for i in range(4):
                            h = SLOT2HEAD[4 * g + i]
                            l, m = h & 1, h // 2
                            nc.tensor.matmul(
                                sc[:kn, i, :qn],
                                kT[64 * l:64 * l + 64, m, t0: t0 + kn],
                                qT[64 * l:64 * l + 64, m, QB * b: QB * b + qn],
                                start=True,
                                stop=True,
                            )
                        nc.scalar.activation(
                            expT[:kn, 4 * g: 4 * g + 4, :qn],
                            sc[:kn, :, :qn],
                            mybir.ActivationFunctionType.Exp,
                            scale=0.125,
                        )
                        nc.vector.tensor_tensor(
                            expT[:kn, 4 * g: 4 * g + 4, :qn],
                            expT[:kn, 4 * g: 4 * g + 4, :qn],
                            band_sb[:kn, None, :qn].broadcast_to([kn, 4, qn]),
                            mybir.AluOpType.mult,
                        )

                # prologue: shingled v 0/1, then block-0 scores
                filler(0, sh_until=1)
                expT_of[0] = spool.tile([128, H, QB], F16, tag="expT",
                                        name="expT_0")
                emit_scores(0, 0)
                emit_scores(0, 1)

                # software-pipelined main loop: block b's AV/denominator/
                # normalize/transpose stages interleaved with block b+1's
                # scores and with v-proj / out-proj PE filler
                for b in range(NBLK):
                    t0, qn, kn = _blk(b)
                    expT = expT_of[b]
                    rinv = spool.tile([QB, H], F32, tag="rinv")
                    attn_sb = spool.tile([QB, 2, 8, HD], F16, tag="attn")
                    if b + 1 < NBLK:
                        expT_of[b + 1] = spool.tile(
                            [128, H, QB], F16, tag="expT",
                            name=f"expT_{b + 1}"
                        )

                    # transpose tile also hosts the denominators in its
                    # tail bytes (bitcast f32) - saves a PSUM bank
                    tr = tr_ps.tile([128, DT * 114 + 2 * H], F16, tag="tr")
                    dn = tr[0:QB, DT * 114: DT * 114 + 2 * H].bitcast(F32)
                    av_tiles = []
                    for half in range(2):
                        for j in range(8):
                            s = 8 * half + j
                            nc.tensor.matmul(
                                dn[:qn, s:s + 1],
                                expT[:kn, s, :qn],
                                ones_sb[:kn, :],
                                start=True,
                                stop=True,
                            )
                        av = av_ps.tile([QB, 8, HD], F32, tag="av")
                        av_tiles.append(av)
                        for j in range(8):
                            s = 8 * half + j
                            h = SLOT2HEAD[s]
                            nc.tensor.matmul(
                                av[:qn, j, :],
                                expT[:kn, s, :qn],
                                v_sb[:kn, b, h, :],
                                start=True,
                                stop=True,
                            )
                        if b + 1 < NBLK:
                            emit_scores(b + 1, half)
                        else:
                            filler(1)

                    # rinv = 1 / (sum - edge_pad_count)
                    nc.vector.tensor_tensor(
                        rinv[:qn],
                        dn[:qn],
                        adj_sb[:qn, b, None].broadcast_to([qn, H]),
                        mybir.AluOpType.subtract,
                    )
                    nc.vector.reciprocal(rinv[:qn], rinv[:qn])

                    # normalize fused into PSUM->SBUF copy
                    for half in range(2):
                        nc.vector.tensor_tensor(
                            attn_sb[:qn, half],
                            av_tiles[half][:qn],
                            rinv[:qn, 8 * half: 8 * half + 8, None].broadcast_to(
                                [qn, 8, HD]
                            ),
                            mybir.AluOpType.mult,
                        )

                    filler(2, sh_until=min(b + 2, NBLK - 1))

                    # attnT via PE transposes (2-slot feature chunks);
                    # slot stride padded to 114 so each fp16 PSUM write
                    # starts 4-byte aligned (walrus verifier rule)
                    tr = tr_ps.tile([128, DT, QB + 1], F16, tag="tr")
                    for half in range(2):
                        for jj in range(4):
                            nc.tensor.transpose(
                                tr[:, 4 * half + jj, :qn],
                                attn_sb[:qn, half, 2 * jj: 2 * jj + 2, :],
                                identity[:qn, :qn],
                            )
                    nc.vector.tensor_copy(
                        attnT[:, :, QB * b: QB * b + qn], tr[:, :, :qn]
                    )
                    done_blocks = b + 1

                # drain: split the last chunks so the final PSUM->SBUF
                # copy + DMA chain is short
                while opj:
                    t, n = opj.pop(0)
                    emit_opj(t, n, cols=256 if not opj else 512)

    nc.compile()
    return nc


def _get_program():
    global _PROGRAM
    if _PROGRAM is None:
        _PROGRAM = _build_program()
    return _PROGRAM


def _host_inputs(x, Wq, Wk, Wv, Wo):
    """Shard + preprocess full inputs into per-core input maps."""
    x = np.asarray(x, dtype=np.float32)
    wts = {}
    for name, w in (("wqT", Wq), ("wkT", Wk), ("wvT", Wv)):
        wts[name] = np.ascontiguousarray(np.asarray(w, np.float32).T).astype(
            np.float16
        )
    # Wo rows permuted to the head-slot order of attnT
    woT = np.ascontiguousarray(np.asarray(Wo, np.float32).T)
    perm = np.array(
        [64 * SLOT2HEAD[f // 64] + f % 64 for f in range(D)], np.int64
    )
    wts["woT"] = woT[perm].astype(np.float16)

    # band mask in [key, q] layout: valid iff q <= k <= q+15
    kk = np.arange(128)[:, None]
    qq = np.arange(QB)[None, :]
    band = ((kk >= qq) & (kk <= qq + WIN - 1)).astype(np.float16)
    ones = np.ones((128, 1), np.float16)

    in_maps = []
    for c in range(NCORES):
        bb, chunk = divmod(c, 4)
        g0 = chunk * CHUNK
        lo, hi = g0 - LP, g0 + CHUNK + RP
        xpad = np.zeros((TH, D), np.float32)
        src_lo, src_hi = max(lo, 0), min(hi, S)
        xpad[src_lo - lo: src_hi - lo] = x[bb, src_lo:src_hi]
        xT = np.ascontiguousarray(xpad.T).astype(np.float16)

        glob = g0 + np.arange(CHUNK)
        pos = glob[:, None] - LP + np.arange(WIN)[None, :]
        counts = ((pos < 0) | (pos >= S)).sum(axis=1).astype(np.float32)
        adj = np.zeros((QB, NBLK), np.float32)
        for b in range(NBLK):
            qn = QB if b < NBLK - 1 else QTAIL
            adj[:qn, b] = counts[QB * b: QB * b + qn]

        in_maps.append(
            {"xT": xT, "adj": adj, "bandT": band, "ones": ones, **wts}
        )
    return in_maps


def kernel(x, Wq, Wk, Wv, Wo):
    global LAST_RESULTS
    nc = _get_program()
    in_maps = _host_inputs(x, Wq, Wk, Wv, Wo)
    res = run_bass_kernel_spmd(
        nc, in_maps, core_ids=list(range(NCORES)), trace=TRACE
    )
    LAST_RESULTS = res
    out = np.empty((B, S, D), np.float32)
    for c in range(NCORES):
        bb, chunk = divmod(c, 4)
        out[bb, chunk * CHUNK:(chunk + 1) * CHUNK] = (
            res.results[c]["out"].astype(np.float32)
        )
    return out


# revision 10
# speedup vs baseline: 1.0207x; 1.0142x over previous
"""Trainium2 Bass kernel for LocalWindowAttention (v2).

Model: B=2, S=4096, D=1024, H=16 heads, hd=64, window 16 (8 left, 7
right), four dim->dim projections (torch-Linear y = x @ W.T), per-token
windowed softmax.

Sharding: 8 cores = 2 batches x 4 sequence chunks of 1024 tokens, each
with a zero-padded halo (8 left / 7 right, 1039 tokens) so K/V at chunk
boundaries are local - no collectives.

Design (driven by the TRN2 instruction-cost model: a matmul costs
out_free_size cycles; stationary loads are free; PE p-state needs
continuous busy):
  - q/k projections in transposed [dout, tok] layout; v in natural
    [tok, dout] layout, computed in 9 ALIGNED 128-token tiles and then
    re-partitioned into 10 SHINGLED key tiles (tile b = tokens
    [113b, 113b+128)) by partition-shifted SBUF->SBUF DMAs, so each
    attention block's AV contraction is a single matmul.
  - attention blocks of QB=113 q tokens: keys span 113+15 = 128.
  - scoresT layout [keys, q] computed directly (stationary kT, moving
    qT, free = q): exp output (SBUF) feeds AV as stationary with no
    probs transpose and no PSUM->SBUF copy.
  - band mask applied as a 0/1 MULTIPLY on the exp output (exp of raw
    scores is fp16-safe), cheap on DVE.
  - denominators via 16 free=1 matmuls (ones moving operand) into a
    [113 q, 16 h] f32 region bitcast into the tail of the transpose
    PSUM tile (saves a PSUM bank): partition-parallel subtract of the
    edge-pad count ("adj") and reciprocal. Padding keys contribute
    exp(0)=1 to the denominator (k=0) and nothing to the numerator
    (v=0), so edge masking is the exact adj subtraction.
  - AV: out[q, hd] natural (stationary expT, moving v, free = hd);
    normalization fused into the PSUM->SBUF copy (multiply by the
    per-partition rinv).
  - attnT via PE transposes (slot stride 114 keeps fp16 PSUM writes
    4-byte aligned); the out projection consumes attnT as stationary.
    Head slots are grouped by PE parity (PSUM tile_position rule), so
    Wo rows are permuted on the host to match the slot order.
  - out-proj results copied PSUM->SBUF fp16 on the Scalar engine and
    DMA'd as fp16; host upcasts.
  - schedule: big-bite DMAs (HWDGE descriptor gen is ~630ns serial per
    DMA) with quarter-bites for the startup-critical wq/xT; q/k
    projections first; v-proj units, k-proj tail and out-proj chunks
    are PE "filler" interleaved into a software-pipelined attention
    loop (block b's AV/normalize/transpose stages overlap block b+1's
    scores/exp) so the PE almost never idles.
"""

import numpy as np

import concourse.bass as bass
import concourse.mybir as mybir
import concourse.tile as tile
from concourse import bacc
from concourse.bass_utils import run_bass_kernel_spmd
from concourse.masks import make_identity

F16 = mybir.dt.float16
F32 = mybir.dt.float32

B, S, D = 2, 4096, 1024
H, HD = 16, 64
WIN, LP, RP = 16, 8, 7
NCORES = 8
CHUNK = S // 4            # tokens per core
TH = CHUNK + LP + RP      # halo token count (1039)
DT = D // 128             # 128-row tiles across D (8)
QB = 113                  # q tokens per attention block (keys = QB+15 = 128)
NBLK = (CHUNK + QB - 1) // QB   # 10 blocks (9 full + 7-token tail)
QTAIL = CHUNK - QB * (NBLK - 1)  # 7
NOUT = CHUNK // 128       # out-proj token tiles (8)

# head slots grouped by PE parity (stationary base partition must be
# uniform within a PSUM bank): slots 0-7 = even heads, 8-15 = odd.
SLOT2HEAD = [0, 2, 4, 6, 8, 10, 12, 14, 1, 3, 5, 7, 9, 11, 13, 15]

TRACE = False             # test.py may set kernel.TRACE = True
LAST_RESULTS = None       # BassKernelResults of the most recent run

_PROGRAM = None


def _blk(b):
    """(t0, qn, kn) for block b: key origin, q count, key count."""
    qn = QB if b < NBLK - 1 else QTAIL
    kn = qn + WIN - 1
    return QB * b, qn, kn


def _build_program():
    nc = bacc.Bacc("TRN2", target_bir_lowering=False, debug=False)

    xT_d = nc.dram_tensor("xT", [D, TH], F16, kind="ExternalInput")
    wq_d = nc.dram_tensor("wqT", [D, D], F16, kind="ExternalInput")
    wk_d = nc.dram_tensor("wkT", [D, D], F16, kind="ExternalInput")
    wv_d = nc.dram_tensor("wvT", [D, D], F16, kind="ExternalInput")
    wo_d = nc.dram_tensor("woT", [D, D], F16, kind="ExternalInput")
    adj_d = nc.dram_tensor("adj", [QB, NBLK], F32, kind="ExternalInput")
    band_d = nc.dram_tensor("bandT", [128, QB], F16, kind="ExternalInput")
    ones_d = nc.dram_tensor("ones", [128, 1], F16, kind="ExternalInput")
    out_d = nc.dram_tensor("out", [CHUNK, D], F16, kind="ExternalOutput")

    with tile.TileContext(nc) as tc:
        with (
            tc.tile_pool(name="const", bufs=1) as cpool,
            tc.tile_pool(name="acts", bufs=1) as apool,
            tc.tile_pool(name="soft", bufs=2) as spool,
            tc.tile_pool(name="outsb", bufs=3) as opool,
        ):
            # ---- constants / activations resident in SBUF ----
            identity = cpool.tile([128, 128], F16)
            make_identity(nc, identity)

            # Weights live whole in SBUF: [128, k, dout]. One or two DMAs
            # each — descriptor generation (HWDGE) is serial and ~630ns
            # per DMA, so few big DMAs beat many small ones.
            xT = apool.tile([128, DT, TH], F16)
            wq_sb = apool.tile([128, DT, D], F16)
            wk_sb = apool.tile([128, DT, D], F16)
            wv_sb = apool.tile([128, DT, D], F16)
            wo_sb = apool.tile([128, DT, D], F16)
            xsrc = xT_d.ap().rearrange("(j p) t -> p j t", p=128)
            # startup-critical: stream wq m-columns and xT tokens in
            # quarter bites so the first q-proj matmuls start ~3us in
            wqsrc = wq_d.ap().rearrange("(j p) o -> p j o", p=128)
            nc.sync.dma_start(wq_sb[:, :, 0:256], wqsrc[:, :, 0:256])
            nc.sync.dma_start(xT[:, :, 0:264], xsrc[:, :, 0:264])
            nc.sync.dma_start(wq_sb[:, :, 256:512], wqsrc[:, :, 256:512])
            nc.sync.dma_start(xT[:, :, 264:520], xsrc[:, :, 264:520])
            nc.sync.dma_start(wq_sb[:, :, 512:D], wqsrc[:, :, 512:D])
            nc.sync.dma_start(xT[:, :, 520:TH], xsrc[:, :, 520:TH])
            nc.sync.dma_start(
                wk_sb, wk_d.ap().rearrange("(j p) o -> p j o", p=128)
            )

            qT = apool.tile([128, DT, CHUNK], F16)
            kT = apool.tile([128, DT, TH], F16)
            v_al = apool.tile([128, 8, H, HD], F16)
            vT_tail = apool.tile([128, DT, 16], F16)
            v_tail = apool.tile([15, D], F16)
            v_sb = apool.tile([128, NBLK, H, HD], F16)
            attnT = apool.tile([128, DT, CHUNK], F16)

            # ---- phase A: q/k projections [dout_tile, tokens] ----
            with (
                tc.tile_pool(name="fill_ps", bufs=2, space="PSUM") as fill_ps,
                tc.tile_pool(name="sc_ps", bufs=2, space="PSUM") as sc_ps,
                tc.tile_pool(name="av_ps", bufs=2, space="PSUM") as av_ps,
                tc.tile_pool(name="tr_ps", bufs=2, space="PSUM") as tr_ps,
            ):
                def qk_chunk(w_sb, dst, m, src_c0, cn, dst_c0):
                    ps = fill_ps.tile([128, 512], F32, tag="fill")
                    for k in range(DT):
                        nc.tensor.matmul(
                            ps[:, :cn],
                            w_sb[:, k, m * 128:(m + 1) * 128],
                            xT[:, k, src_c0: src_c0 + cn],
                            start=(k == 0),
                            stop=(k == DT - 1),
                        )
                    nc.vector.tensor_copy(dst[:, m, dst_c0:dst_c0 + cn], ps[:, :cn])

                # PE warm-up: the p-state ramp reaches full speed only
                # after 3us of continuous busy, so chew on the identity
                # tile while the first wq/xT DMAs stream. Results land in
                # a scratch PSUM tile nobody reads.
                warm = fill_ps.tile([128, 512], F32, tag="fill")
                for _ in range(44):
                    nc.tensor.matmul(
                        warm[:, 0:128], identity, identity,
                        start=True, stop=True,
                    )

                # q-proj: tokens [LP, LP+1024); follow the quarter-bite
                # DMA stream: m 0-1 on 256-token chunks first
                for m in range(2):
                    qk_chunk(wq_sb, qT, m, LP, 256, 0)
                for m in range(2):
                    qk_chunk(wq_sb, qT, m, LP + 256, 256, 256)
                for m in range(2, 4):
                    qk_chunk(wq_sb, qT, m, LP, 512, 0)
                ones_sb = cpool.tile([128, 1], F16)
                band_sb = cpool.tile([128, QB], F16)
                adj_sb = cpool.tile([QB, NBLK], F32)
                nc.sync.dma_start(ones_sb, ones_d.ap())
                nc.sync.dma_start(band_sb, band_d.ap())
                nc.sync.dma_start(adj_sb, adj_d.ap())
                nc.sync.dma_start(
                    wv_sb, wv_d.ap().rearrange("(j p) o -> p j o", p=128)
                )
                for m in range(4, DT):
                    qk_chunk(wq_sb, qT, m, LP, 512, 0)
                for m in range(DT):
                    qk_chunk(wq_sb, qT, m, LP + 512, 512, 512)

                # k-proj main chunks over [0, 1024)
                for m in range(DT):
                    qk_chunk(wk_sb, kT, m, 0, 512, 0)
                nc.sync.dma_start(
                    wo_sb, wo_d.ap().rearrange("(j p) o -> p j o", p=128)
                )
                for m in range(DT):
                    qk_chunk(wk_sb, kT, m, 512, 512, 512)

                # k-proj tail [1024, 1039): all 8 m-tiles batched into one
                # PSUM tile (8 accumulation groups, 16-col aligned slots),
                # one copy out
                kt = fill_ps.tile([128, DT, 16], F32, tag="fill")
                for m in range(DT):
                    for k in range(DT):
                        nc.tensor.matmul(
                            kt[:, m, 0:TH - 1024],
                            wk_sb[:, k, m * 128:(m + 1) * 128],
                            xT[:, k, 1024:TH],
                            start=(k == 0),
                            stop=(k == DT - 1),
                        )
                nc.vector.tensor_copy(
                    kT[:, :, 1024:TH], kt[:, :, 0:TH - 1024]
                )

                # ---- phase B: v-proj + attention + out-proj ----
                # filler units: one PSUM-group of 8 matmuls each.
                # aligned-v units first (with shingle DMAs draining as
                # sources complete), then out-proj chunks.
                vq = [(a, n) for a in range(8) for n in range(2)]
                shq = list(range(NBLK))
                opj = [(t, n) for t in range(NOUT) for n in range(2)]
                done_blocks = 0
                v_done = -1

                def shingle_src_max(b):
                    t0, qn, kn = _blk(b)
                    a, off = divmod(QB * b, 128)
                    # tokens past 1024 come from v_tail (prologue), so the
                    # aligned-tile requirement caps at tile 7
                    return min(a if kn <= 128 - off else a + 1, 7)

                def emit_v(a, n):
                    # aligned v tile a = tokens [128a, min(128a+128, TH))
                    rows = min(128, TH - 128 * a)
                    ps = fill_ps.tile([128, 512], F32, tag="fill")
                    for k in range(DT):
                        nc.tensor.matmul(
                            ps[:rows, :],
                            xT[:, k, 128 * a: 128 * a + rows],
                            wv_sb[:, k, n * 512:(n + 1) * 512],
                            start=(k == 0),
                            stop=(k == DT - 1),
                        )
                    nc.vector.tensor_copy(
                        v_al[:rows, a, 8 * n: 8 * n + 8, :], ps[:rows, :]
                    )

                def emit_shingle(b):
                    # shingled key tile b = tokens [113b, 113b+kn) copied
                    # out of the aligned tiles (partition-shifted SBUF DMA);
                    # rows past token 1024 come from the transposed tail
                    t0, qn, kn = _blk(b)
                    a, off = divmod(QB * b, 128)
                    len1 = min(128 - off, kn, 1024 - QB * b)
                    nc.sync.dma_start(
                        v_sb[0:len1, b], v_al[off: off + len1, a]
                    )
                    if QB * b + kn > 1024:
                        nt = QB * b + kn - 1024
                        nc.sync.dma_start(
                            v_sb[kn - nt: kn, b],
                            v_tail[0:nt].rearrange(
                                "p (h d) -> p h d", h=H),
                        )
                    elif kn > len1:
                        nc.sync.dma_start(
                            v_sb[len1:kn, b], v_al[0: kn - len1, a + 1]
                        )

                def emit_opj(t, n, cols=512):
                    for c0 in range(n * 512, (n + 1) * 512, cols):
                        ps = fill_ps.tile([128, 512], F32, tag="fill")
                        for k in range(DT):
                            nc.tensor.matmul(
                                ps[:, :cols],
                                attnT[:, k, t * 128:(t + 1) * 128],
                                wo_sb[:, k, c0: c0 + cols],
                                start=(k == 0),
                                stop=(k == DT - 1),
                            )
                        osb = opool.tile([128, 512], F16, tag="osb")
                        nc.scalar.copy(osb[:, :cols], ps[:, :cols])
                        nc.sync.dma_start(
                            out_d.ap()[t * 128:(t + 1) * 128, c0: c0 + cols],
                            osb[:, :cols],
                        )

                def pop_v():
                    nonlocal v_done
                    a, n = vq.pop(0)
                    emit_v(a, n)
                    if n == 1:
                        v_done = a
                    while shq and shingle_src_max(shq[0]) <= v_done:
                        emit_shingle(shq.pop(0))

                def filler(budget, sh_until=-1):
                    """Emit filler PE work: aligned-v units until blocks
                    <= sh_until have their shingled v, then up to `budget`
                    units total from the v queue and ready out-proj."""
                    emitted = 0
                    while shq and shq[0] <= sh_until:
                        pop_v()
                        emitted += 1
                    while emitted < budget:
                        if vq:
                            pop_v()
                        elif opj:
                            t, n = opj[0]
                            need = -(-(128 * (t + 1)) // QB)  # ceil
                            if done_blocks < need:
                                break
                            opj.pop(0)
                            emit_opj(t, n)
                        else:
                            break
                        emitted += 1

                expT_of = {}

                def emit_scores(b, gh):
                    """Scores+exp+band for groups (2gh, 2gh+1) of block b."""
                    t0, qn, kn = _blk(b)
                    expT = expT_of[b]
                    for g in (2 * gh, 2 * gh + 1):
                        sc = sc_ps.tile([128, 4, QB], F32, tag="sc")
                        for i in range(4):
                            h = SLOT2HEAD[4 * g + i]
                            l, m = h & 1, h // 2
                            nc.tensor.matmul(
                                sc[:kn, i, :qn],
                                kT[64 * l:64 * l + 64, m, t0: t0 + kn],
                                qT[64 * l:64 * l + 64, m, QB * b: QB * b + qn],
                                start=True,
                                stop=True,
                            )
                        nc.scalar.activation(
                            expT[:kn, 4 * g: 4 * g + 4, :qn],
                            sc[:kn, :, :qn],
                            mybir.ActivationFunctionType.Exp,
                            scale=0.125,
                        )
                        nc.vector.tensor_tensor(
                            expT[:kn, 4 * g: 4 * g + 4, :qn],
                            expT[:kn, 4 * g: 4 * g + 4, :qn],
                            band_sb[:kn, None, :qn].broadcast_to([kn, 4, qn]),
                            mybir.AluOpType.mult,
                        )

                # v tail [1024, 1039): 15 tokens cost a full 512-free
                # tile in natural layout, but only free=15 in transposed
                # layout; compute vT and transpose back through the PE.
                vt = fill_ps.tile([128, DT, 16], F32, tag="fill")
                for m in range(DT):
                    for k in range(DT):
                        nc.tensor.matmul(
                            vt[:, m, 0:15],
                            wv_sb[:, k, m * 128:(m + 1) * 128],
                            xT[:, k, 1024:TH],
                            start=(k == 0),
                            stop=(k == DT - 1),
                        )
                nc.vector.tensor_copy(vT_tail[:, :, 0:15], vt[:, :, 0:15])
                vtp32 = fill_ps.tile([128, 512], F32, tag="fill")
                vtp = vtp32.bitcast(F16)  # [128, 1024] f16 view
                for m in range(DT):
                    nc.tensor.transpose(
                        vtp[0:15, m * 128:(m + 1) * 128],
                        vT_tail[:, m, 0:15],
                        identity,
                    )
                nc.vector.tensor_copy(v_tail, vtp[0:15, :])

                # prologue: shingled v 0/1, then block-0 scores
                filler(0, sh_until=1)
                expT_of[0] = spool.tile([128, H, QB], F16, tag="expT",
                                        name="expT_0")
                emit_scores(0, 0)
                emit_scores(0, 1)

                # software-pipelined main loop: block b's AV/denominator/
                # normalize/transpose stages interleaved with block b+1's
                # scores and with v-proj / out-proj PE filler
                for b in range(NBLK):
                    t0, qn, kn = _blk(b)
                    expT = expT_of[b]
                    rinv = spool.tile([QB, H], F32, tag="rinv")
                    attn_sb = spool.tile([QB, 2, 8, HD], F16, tag="attn")
                    if b + 1 < NBLK:
                        expT_of[b + 1] = spool.tile(
                            [128, H, QB], F16, tag="expT",
                            name=f"expT_{b + 1}"
                        )

                    # transpose tile also hosts the denominators in its
                    # tail bytes (bitcast f32) - saves a PSUM bank
                    tr = tr_ps.tile([128, DT * 114 + 2 * H], F16, tag="tr")
                    dn = tr[0:QB, DT * 114: DT * 114 + 2 * H].bitcast(F32)
                    av_tiles = []
                    for half in range(2):
                        for j in range(8):
                            s = 8 * half + j
                            nc.tensor.matmul(
                                dn[:qn, s:s + 1],
                                expT[:kn, s, :qn],
                                ones_sb[:kn, :],
                                start=True,
                                stop=True,
                            )
                        av = av_ps.tile([QB, 8, HD], F32, tag="av")
                        av_tiles.append(av)
                        for j in range(8):
                            s = 8 * half + j
                            h = SLOT2HEAD[s]
                            nc.tensor.matmul(
                                av[:qn, j, :],
                                expT[:kn, s, :qn],
                                v_sb[:kn, b, h, :],
                                start=True,
                                stop=True,
                            )
                        if b + 1 < NBLK:
                            emit_scores(b + 1, half)
                        else:
                            filler(1)

                    # rinv = 1 / (sum - edge_pad_count)
                    nc.vector.tensor_tensor(
                        rinv[:qn],
                        dn[:qn],
                        adj_sb[:qn, b, None].broadcast_to([qn, H]),
                        mybir.AluOpType.subtract,
                    )
                    nc.vector.reciprocal(rinv[:qn], rinv[:qn])

                    # normalize fused into PSUM->SBUF copy
                    for half in range(2):
                        nc.vector.tensor_tensor(
                            attn_sb[:qn, half],
                            av_tiles[half][:qn],
                            rinv[:qn, 8 * half: 8 * half + 8, None].broadcast_to(
                                [qn, 8, HD]
                            ),
                            mybir.AluOpType.mult,
                        )

                    filler(2, sh_until=min(b + 2, NBLK - 1))

                    # attnT via PE transposes (2-slot feature chunks);
                    # slot stride padded to 114 so each fp16 PSUM write
                    # starts 4-byte aligned (walrus verifier rule)
                    tr = tr_ps.tile([128, DT, QB + 1], F16, tag="tr")
                    for half in range(2):
                        for jj in range(4):
                            nc.tensor.transpose(
                                tr[:, 4 * half + jj, :qn],
                                attn_sb[:qn, half, 2 * jj: 2 * jj + 2, :],
                                identity[:qn, :qn],
                            )
                    nc.vector.tensor_copy(
                        attnT[:, :, QB * b: QB * b + qn], tr[:, :, :qn]
                    )
                    done_blocks = b + 1

                # drain: split the last chunks so the final PSUM->SBUF
                # copy + DMA chain is short
                while opj:
                    t, n = opj.pop(0)
                    emit_opj(t, n, cols=256 if not opj else 512)

    nc.compile()
    return nc


def _get_program():
    global _PROGRAM
    if _PROGRAM is None:
        _PROGRAM = _build_program()
    return _PROGRAM


def _host_inputs(x, Wq, Wk, Wv, Wo):
    """Shard + preprocess full inputs into per-core input maps."""
    x = np.asarray(x, dtype=np.float32)
    wts = {}
    for name, w in (("wqT", Wq), ("wkT", Wk), ("wvT", Wv)):
        wts[name] = np.ascontiguousarray(np.asarray(w, np.float32).T).astype(
            np.float16
        )
    # Wo rows permuted to the head-slot order of attnT
    woT = np.ascontiguousarray(np.asarray(Wo, np.float32).T)
    perm = np.array(
        [64 * SLOT2HEAD[f // 64] + f % 64 for f in range(D)], np.int64
    )
    wts["woT"] = woT[perm].astype(np.float16)

    # band mask in [key, q] layout: valid iff q <= k <= q+15
    kk = np.arange(128)[:, None]
    qq = np.arange(QB)[None, :]
    band = ((kk >= qq) & (kk <= qq + WIN - 1)).astype(np.float16)
    ones = np.ones((128, 1), np.float16)

    in_maps = []
    for c in range(NCORES):
        bb, chunk = divmod(c, 4)
        g0 = chunk * CHUNK
        lo, hi = g0 - LP, g0 + CHUNK + RP
        xpad = np.zeros((TH, D), np.float32)
        src_lo, src_hi = max(lo, 0), min(hi, S)
        xpad[src_lo - lo: src_hi - lo] = x[bb, src_lo:src_hi]
        xT = np.ascontiguousarray(xpad.T).astype(np.float16)

        glob = g0 + np.arange(CHUNK)
        pos = glob[:, None] - LP + np.arange(WIN)[None, :]
        counts = ((pos < 0) | (pos >= S)).sum(axis=1).astype(np.float32)
        adj = np.zeros((QB, NBLK), np.float32)
        for b in range(NBLK):
            qn = QB if b < NBLK - 1 else QTAIL
            adj[:qn, b] = counts[QB * b: QB * b + qn]

        in_maps.append(
            {"xT": xT, "adj": adj, "bandT": band, "ones": ones, **wts}
        )
    return in_maps


def kernel(x, Wq, Wk, Wv, Wo):
    global LAST_RESULTS
    nc = _get_program()
    in_maps = _host_inputs(x, Wq, Wk, Wv, Wo)
    res = run_bass_kernel_spmd(
        nc, in_maps, core_ids=list(range(NCORES)), trace=TRACE
    )
    LAST_RESULTS = res
    out = np.empty((B, S, D), np.float32)
    for c in range(NCORES):
        bb, chunk = divmod(c, 4)
        out[bb, chunk * CHUNK:(chunk + 1) * CHUNK] = (
            res.results[c]["out"].astype(np.float32)
        )
    return out


# revision 11
# speedup vs baseline: 1.0297x; 1.0088x over previous
"""Trainium2 Bass kernel for LocalWindowAttention (v2).

Model: B=2, S=4096, D=1024, H=16 heads, hd=64, window 16 (8 left, 7
right), four dim->dim projections (torch-Linear y = x @ W.T), per-token
windowed softmax.

Sharding: 8 cores = 2 batches x 4 sequence chunks of 1024 tokens, each
with a zero-padded halo (8 left / 7 right, 1039 tokens) so K/V at chunk
boundaries are local - no collectives.

Design (driven by the TRN2 instruction-cost model: a matmul costs
out_free_size cycles; stationary loads are free; PE p-state needs
continuous busy):
  - q/k projections in transposed [dout, tok] layout; v in natural
    [tok, dout] layout, computed in 9 ALIGNED 128-token tiles and then
    re-partitioned into 10 SHINGLED key tiles (tile b = tokens
    [113b, 113b+128)) by partition-shifted SBUF->SBUF DMAs, so each
    attention block's AV contraction is a single matmul.
  - attention blocks of QB=113 q tokens: keys span 113+15 = 128.
  - scoresT layout [keys, q] computed directly (stationary kT, moving
    qT, free = q): exp output (SBUF) feeds AV as stationary with no
    probs transpose and no PSUM->SBUF copy.
  - band mask applied as a 0/1 MULTIPLY on the exp output (exp of raw
    scores is fp16-safe), cheap on DVE.
  - denominators via 16 free=1 matmuls (ones moving operand) into a
    [113 q, 16 h] f32 region bitcast into the tail of the transpose
    PSUM tile (saves a PSUM bank): partition-parallel subtract of the
    edge-pad count ("adj") and reciprocal. Padding keys contribute
    exp(0)=1 to the denominator (k=0) and nothing to the numerator
    (v=0), so edge masking is the exact adj subtraction.
  - AV: out[q, hd] natural (stationary expT, moving v, free = hd);
    normalization fused into the PSUM->SBUF copy (multiply by the
    per-partition rinv).
  - attnT via PE transposes (slot stride 114 keeps fp16 PSUM writes
    4-byte aligned); the out projection consumes attnT as stationary.
    Head slots are grouped by PE parity (PSUM tile_position rule), so
    Wo rows are permuted on the host to match the slot order.
  - out-proj results copied PSUM->SBUF fp16 on the Scalar engine and
    DMA'd as fp16; host upcasts.
  - schedule: big-bite DMAs (HWDGE descriptor gen is ~630ns serial per
    DMA) with quarter-bites for the startup-critical wq/xT; q/k
    projections first; v-proj units, k-proj tail and out-proj chunks
    are PE "filler" interleaved into a software-pipelined attention
    loop (block b's AV/normalize/transpose stages overlap block b+1's
    scores/exp) so the PE almost never idles.
"""

import numpy as np

import concourse.bass as bass
import concourse.mybir as mybir
import concourse.tile as tile
from concourse import bacc
from concourse.bass_utils import run_bass_kernel_spmd
from concourse.masks import make_identity

F16 = mybir.dt.float16
F32 = mybir.dt.float32

B, S, D = 2, 4096, 1024
H, HD = 16, 64
WIN, LP, RP = 16, 8, 7
NCORES = 8
CHUNK = S // 4            # tokens per core
TH = CHUNK + LP + RP      # halo token count (1039)
DT = D // 128             # 128-row tiles across D (8)
QB = 113                  # q tokens per attention block (keys = QB+15 = 128)
NBLK = (CHUNK + QB - 1) // QB   # 10 blocks (9 full + 7-token tail)
QTAIL = CHUNK - QB * (NBLK - 1)  # 7
NOUT = CHUNK // 128       # out-proj token tiles (8)

# head slots grouped by PE parity (stationary base partition must be
# uniform within a PSUM bank): slots 0-7 = even heads, 8-15 = odd.
SLOT2HEAD = [0, 2, 4, 6, 8, 10, 12, 14, 1, 3, 5, 7, 9, 11, 13, 15]

TRACE = False             # test.py may set kernel.TRACE = True
LAST_RESULTS = None       # BassKernelResults of the most recent run

_PROGRAM = None


def _blk(b):
    """(t0, qn, kn) for block b: key origin, q count, key count."""
    qn = QB if b < NBLK - 1 else QTAIL
    kn = qn + WIN - 1
    return QB * b, qn, kn


def _build_program():
    nc = bacc.Bacc("TRN2", target_bir_lowering=False, debug=False)

    xT_d = nc.dram_tensor("xT", [D, TH], F16, kind="ExternalInput")
    wq_d = nc.dram_tensor("wqT", [D, D], F16, kind="ExternalInput")
    wk_d = nc.dram_tensor("wkT", [D, D], F16, kind="ExternalInput")
    wv_d = nc.dram_tensor("wvT", [D, D], F16, kind="ExternalInput")
    wo_d = nc.dram_tensor("woT", [D, D], F16, kind="ExternalInput")
    adj_d = nc.dram_tensor("adj", [QB, NBLK], F32, kind="ExternalInput")
    band_d = nc.dram_tensor("bandT", [128, QB], F16, kind="ExternalInput")
    ones_d = nc.dram_tensor("ones", [128, 1], F16, kind="ExternalInput")
    out_d = nc.dram_tensor("out", [CHUNK, D], F16, kind="ExternalOutput")

    with tile.TileContext(nc) as tc:
        with (
            tc.tile_pool(name="const", bufs=1) as cpool,
            tc.tile_pool(name="acts", bufs=1) as apool,
            tc.tile_pool(name="soft", bufs=2) as spool,
            tc.tile_pool(name="outsb", bufs=3) as opool,
        ):
            # ---- constants / activations resident in SBUF ----
            identity = cpool.tile([128, 128], F16)
            make_identity(nc, identity)

            # Weights live whole in SBUF: [128, k, dout]. One or two DMAs
            # each — descriptor generation (HWDGE) is serial and ~630ns
            # per DMA, so few big DMAs beat many small ones.
            xT = apool.tile([128, DT, TH], F16)
            wq_sb = apool.tile([128, DT, D], F16)
            wk_sb = apool.tile([128, DT, D], F16)
            wv_sb = apool.tile([128, DT, D], F16)
            wo_sb = apool.tile([128, DT, D], F16)
            xsrc = xT_d.ap().rearrange("(j p) t -> p j t", p=128)
            # startup-critical: stream wq m-columns and xT tokens in
            # quarter bites so the first q-proj matmuls start ~3us in
            wqsrc = wq_d.ap().rearrange("(j p) o -> p j o", p=128)
            nc.sync.dma_start(wq_sb[:, :, 0:256], wqsrc[:, :, 0:256])
            nc.sync.dma_start(xT[:, :, 0:264], xsrc[:, :, 0:264])
            nc.sync.dma_start(xT[:, :, 264:520], xsrc[:, :, 264:520])
            nc.sync.dma_start(wq_sb[:, :, 256:512], wqsrc[:, :, 256:512])
            nc.sync.dma_start(wq_sb[:, :, 512:D], wqsrc[:, :, 512:D])
            nc.sync.dma_start(xT[:, :, 520:TH], xsrc[:, :, 520:TH])
            nc.sync.dma_start(
                wk_sb, wk_d.ap().rearrange("(j p) o -> p j o", p=128)
            )

            qT = apool.tile([128, DT, CHUNK], F16)
            kT = apool.tile([128, DT, TH], F16)
            v_al = apool.tile([128, 8, H, HD], F16)
            vT_tail = apool.tile([128, DT, 16], F16)
            v_tail = apool.tile([15, D], F16)
            v_sb = apool.tile([128, NBLK, H, HD], F16)
            attnT = apool.tile([128, DT, CHUNK], F16)

            # ---- phase A: q/k projections [dout_tile, tokens] ----
            with (
                tc.tile_pool(name="fill_ps", bufs=2, space="PSUM") as fill_ps,
                tc.tile_pool(name="sc_ps", bufs=2, space="PSUM") as sc_ps,
                tc.tile_pool(name="av_ps", bufs=2, space="PSUM") as av_ps,
                tc.tile_pool(name="tr_ps", bufs=2, space="PSUM") as tr_ps,
            ):
                def qk_chunk(w_sb, dst, m, src_c0, cn, dst_c0):
                    ps = fill_ps.tile([128, 512], F32, tag="fill")
                    for k in range(DT):
                        nc.tensor.matmul(
                            ps[:, :cn],
                            w_sb[:, k, m * 128:(m + 1) * 128],
                            xT[:, k, src_c0: src_c0 + cn],
                            start=(k == 0),
                            stop=(k == DT - 1),
                        )
                    nc.vector.tensor_copy(dst[:, m, dst_c0:dst_c0 + cn], ps[:, :cn])

                # PE warm-up: the p-state ramp reaches full speed only
                # after 3us of continuous busy, so chew on the identity
                # tile while the first wq/xT DMAs stream. Results land in
                # a scratch PSUM tile nobody reads.
                warm = fill_ps.tile([128, 512], F32, tag="fill")
                for _ in range(44):
                    nc.tensor.matmul(
                        warm[:, 0:128], identity, identity,
                        start=True, stop=True,
                    )

                # q-proj: tokens [LP, LP+1024); follow the quarter-bite
                # DMA stream: m 0-1 on 256-token chunks first
                for m in range(2):
                    qk_chunk(wq_sb, qT, m, LP, 256, 0)
                for m in range(2):
                    qk_chunk(wq_sb, qT, m, LP + 256, 256, 256)
                for m in range(2, 4):
                    qk_chunk(wq_sb, qT, m, LP, 512, 0)
                ones_sb = cpool.tile([128, 1], F16)
                band_sb = cpool.tile([128, QB], F16)
                adj_sb = cpool.tile([QB, NBLK], F32)
                nc.sync.dma_start(ones_sb, ones_d.ap())
                nc.sync.dma_start(band_sb, band_d.ap())
                nc.sync.dma_start(adj_sb, adj_d.ap())
                nc.sync.dma_start(
                    wv_sb, wv_d.ap().rearrange("(j p) o -> p j o", p=128)
                )
                for m in range(4, DT):
                    qk_chunk(wq_sb, qT, m, LP, 512, 0)
                for m in range(DT):
                    qk_chunk(wq_sb, qT, m, LP + 512, 512, 512)

                # k-proj main chunks over [0, 1024)
                for m in range(DT):
                    qk_chunk(wk_sb, kT, m, 0, 512, 0)
                nc.sync.dma_start(
                    wo_sb, wo_d.ap().rearrange("(j p) o -> p j o", p=128)
                )
                for m in range(DT):
                    qk_chunk(wk_sb, kT, m, 512, 512, 512)

                # k-proj tail [1024, 1039): all 8 m-tiles batched into one
                # PSUM tile (8 accumulation groups, 16-col aligned slots),
                # one copy out
                kt = fill_ps.tile([128, DT, 16], F32, tag="fill")
                for m in range(DT):
                    for k in range(DT):
                        nc.tensor.matmul(
                            kt[:, m, 0:TH - 1024],
                            wk_sb[:, k, m * 128:(m + 1) * 128],
                            xT[:, k, 1024:TH],
                            start=(k == 0),
                            stop=(k == DT - 1),
                        )
                nc.vector.tensor_copy(
                    kT[:, :, 1024:TH], kt[:, :, 0:TH - 1024]
                )

                # ---- phase B: v-proj + attention + out-proj ----
                # filler units: one PSUM-group of 8 matmuls each.
                # aligned-v units first (with shingle DMAs draining as
                # sources complete), then out-proj chunks.
                vq = [(a, n) for a in range(8) for n in range(2)]
                shq = list(range(NBLK))
                opj = [(t, n) for t in range(NOUT) for n in range(2)]
                done_blocks = 0
                v_done = -1

                def shingle_src_max(b):
                    t0, qn, kn = _blk(b)
                    a, off = divmod(QB * b, 128)
                    # tokens past 1024 come from v_tail (prologue), so the
                    # aligned-tile requirement caps at tile 7
                    return min(a if kn <= 128 - off else a + 1, 7)

                def emit_v(a, n):
                    # aligned v tile a = tokens [128a, min(128a+128, TH))
                    rows = min(128, TH - 128 * a)
                    ps = fill_ps.tile([128, 512], F32, tag="fill")
                    for k in range(DT):
                        nc.tensor.matmul(
                            ps[:rows, :],
                            xT[:, k, 128 * a: 128 * a + rows],
                            wv_sb[:, k, n * 512:(n + 1) * 512],
                            start=(k == 0),
                            stop=(k == DT - 1),
                        )
                    nc.vector.tensor_copy(
                        v_al[:rows, a, 8 * n: 8 * n + 8, :], ps[:rows, :]
                    )

                def emit_shingle(b):
                    # shingled key tile b = tokens [113b, 113b+kn) copied
                    # out of the aligned tiles (partition-shifted SBUF DMA);
                    # rows past token 1024 come from the transposed tail
                    t0, qn, kn = _blk(b)
                    a, off = divmod(QB * b, 128)
                    len1 = min(128 - off, kn, 1024 - QB * b)
                    nc.sync.dma_start(
                        v_sb[0:len1, b], v_al[off: off + len1, a]
                    )
                    if QB * b + kn > 1024:
                        nt = QB * b + kn - 1024
                        nc.sync.dma_start(
                            v_sb[kn - nt: kn, b],
                            v_tail[0:nt].rearrange(
                                "p (h d) -> p h d", h=H),
                        )
                    elif kn > len1:
                        nc.sync.dma_start(
                            v_sb[len1:kn, b], v_al[0: kn - len1, a + 1]
                        )

                def emit_opj(t, n, cols=512):
                    for c0 in range(n * 512, (n + 1) * 512, cols):
                        ps = fill_ps.tile([128, 512], F32, tag="fill")
                        for k in range(DT):
                            nc.tensor.matmul(
                                ps[:, :cols],
                                attnT[:, k, t * 128:(t + 1) * 128],
                                wo_sb[:, k, c0: c0 + cols],
                                start=(k == 0),
                                stop=(k == DT - 1),
                            )
                        osb = opool.tile([128, 512], F16, tag="osb")
                        nc.scalar.copy(osb[:, :cols], ps[:, :cols])
                        nc.sync.dma_start(
                            out_d.ap()[t * 128:(t + 1) * 128, c0: c0 + cols],
                            osb[:, :cols],
                        )

                def pop_v():
                    nonlocal v_done
                    a, n = vq.pop(0)
                    emit_v(a, n)
                    if n == 1:
                        v_done = a
                    while shq and shingle_src_max(shq[0]) <= v_done:
                        emit_shingle(shq.pop(0))

                def filler(budget, sh_until=-1):
                    """Emit filler PE work: aligned-v units until blocks
                    <= sh_until have their shingled v, then up to `budget`
                    units total from the v queue and ready out-proj."""
                    emitted = 0
                    while shq and shq[0] <= sh_until:
                        pop_v()
                        emitted += 1
                    while emitted < budget:
                        if vq:
                            pop_v()
                        elif opj:
                            t, n = opj[0]
                            need = -(-(128 * (t + 1)) // QB)  # ceil
                            if done_blocks < need:
                                break
                            opj.pop(0)
                            emit_opj(t, n)
                        else:
                            break
                        emitted += 1

                expT_of = {}

                def emit_scores(b, gh):
                    """Scores+exp+band for groups (2gh, 2gh+1) of block b."""
                    t0, qn, kn = _blk(b)
                    expT = expT_of[b]
                    for g in (2 * gh, 2 * gh + 1):
                        sc = sc_ps.tile([128, 4, QB], F32, tag="sc")
                        for i in range(4):
                            h = SLOT2HEAD[4 * g + i]
                            l, m = h & 1, h // 2
                            nc.tensor.matmul(
                                sc[:kn, i, :qn],
                                kT[64 * l:64 * l + 64, m, t0: t0 + kn],
                                qT[64 * l:64 * l + 64, m, QB * b: QB * b + qn],
                                start=True,
                                stop=True,
                            )
                        nc.scalar.activation(
                            expT[:kn, 4 * g: 4 * g + 4, :qn],
                            sc[:kn, :, :qn],
                            mybir.ActivationFunctionType.Exp,
                            scale=0.125,
                        )
                        nc.vector.tensor_tensor(
                            expT[:kn, 4 * g: 4 * g + 4, :qn],
                            expT[:kn, 4 * g: 4 * g + 4, :qn],
                            band_sb[:kn, None, :qn].broadcast_to([kn, 4, qn]),
                            mybir.AluOpType.mult,
                        )

                # v tail [1024, 1039): 15 tokens cost a full 512-free
                # tile in natural layout, but only free=15 in transposed
                # layout; compute vT and transpose back through the PE.
                vt = fill_ps.tile([128, DT, 16], F32, tag="fill")
                for m in range(DT):
                    for k in range(DT):
                        nc.tensor.matmul(
                            vt[:, m, 0:15],
                            wv_sb[:, k, m * 128:(m + 1) * 128],
                            xT[:, k, 1024:TH],
                            start=(k == 0),
                            stop=(k == DT - 1),
                        )
                nc.vector.tensor_copy(vT_tail[:, :, 0:15], vt[:, :, 0:15])
                vtp32 = fill_ps.tile([128, 512], F32, tag="fill")
                vtp = vtp32.bitcast(F16)  # [128, 1024] f16 view
                for m in range(DT):
                    nc.tensor.transpose(
                        vtp[0:15, m * 128:(m + 1) * 128],
                        vT_tail[:, m, 0:15],
                        identity,
                    )
                nc.vector.tensor_copy(v_tail, vtp[0:15, :])

                # prologue: shingled v 0/1, then block-0 scores
                filler(0, sh_until=1)
                expT_of[0] = spool.tile([128, H, QB], F16, tag="expT",
                                        name="expT_0")
                emit_scores(0, 0)
                emit_scores(0, 1)

                # software-pipelined main loop: block b's AV/denominator/
                # normalize/transpose stages interleaved with block b+1's
                # scores and with v-proj / out-proj PE filler
                for b in range(NBLK):
                    t0, qn, kn = _blk(b)
                    expT = expT_of[b]
                    rinv = spool.tile([QB, H], F32, tag="rinv")
                    attn_sb = spool.tile([QB, 2, 8, HD], F16, tag="attn")
                    if b + 1 < NBLK:
                        expT_of[b + 1] = spool.tile(
                            [128, H, QB], F16, tag="expT",
                            name=f"expT_{b + 1}"
                        )

                    # transpose tile also hosts the denominators in its
                    # tail bytes (bitcast f32) - saves a PSUM bank
                    tr = tr_ps.tile([128, DT * 114 + 2 * H], F16, tag="tr")
                    dn = tr[0:QB, DT * 114: DT * 114 + 2 * H].bitcast(F32)
                    av_tiles = []
                    for half in range(2):
                        for j in range(8):
                            s = 8 * half + j
                            nc.tensor.matmul(
                                dn[:qn, s:s + 1],
                                expT[:kn, s, :qn],
                                ones_sb[:kn, :],
                                start=True,
                                stop=True,
                            )
                        av = av_ps.tile([QB, 8, HD], F32, tag="av")
                        av_tiles.append(av)
                        for j in range(8):
                            s = 8 * half + j
                            h = SLOT2HEAD[s]
                            nc.tensor.matmul(
                                av[:qn, j, :],
                                expT[:kn, s, :qn],
                                v_sb[:kn, b, h, :],
                                start=True,
                                stop=True,
                            )
                        if b + 1 < NBLK:
                            emit_scores(b + 1, half)
                        else:
                            filler(1)

                    # rinv = 1 / (sum - edge_pad_count)
                    nc.vector.tensor_tensor(
                        rinv[:qn],
                        dn[:qn],
                        adj_sb[:qn, b, None].broadcast_to([qn, H]),
                        mybir.AluOpType.subtract,
                    )
                    nc.vector.reciprocal(rinv[:qn], rinv[:qn])

                    # normalize fused into PSUM->SBUF copy
                    for half in range(2):
                        nc.vector.tensor_tensor(
                            attn_sb[:qn, half],
                            av_tiles[half][:qn],
                            rinv[:qn, 8 * half: 8 * half + 8, None].broadcast_to(
                                [qn, 8, HD]
                            ),
                            mybir.AluOpType.mult,
                        )

                    filler(2, sh_until=min(b + 2, NBLK - 1))

                    # attnT via PE transposes (2-slot feature chunks);
                    # slot stride padded to 114 so each fp16 PSUM write
                    # starts 4-byte aligned (walrus verifier rule)
                    tr = tr_ps.tile([128, DT, QB + 1], F16, tag="tr")
                    for half in range(2):
                        for jj in range(4):
                            nc.tensor.transpose(
                                tr[:, 4 * half + jj, :qn],
                                attn_sb[:qn, half, 2 * jj: 2 * jj + 2, :],
                                identity[:qn, :qn],
                            )
                    nc.vector.tensor_copy(
                        attnT[:, :, QB * b: QB * b + qn], tr[:, :, :qn]
                    )
                    done_blocks = b + 1

                # drain: split the last chunks so the final PSUM->SBUF
                # copy + DMA chain is short
                while opj:
                    t, n = opj.pop(0)
                    emit_opj(t, n, cols=256 if not opj else 512)

    nc.compile()
    return nc


def _get_program():
    global _PROGRAM
    if _PROGRAM is None:
        _PROGRAM = _build_program()
    return _PROGRAM


def _host_inputs(x, Wq, Wk, Wv, Wo):
    """Shard + preprocess full inputs into per-core input maps."""
    x = np.asarray(x, dtype=np.float32)
    wts = {}
    for name, w in (("wqT", Wq), ("wkT", Wk), ("wvT", Wv)):
        wts[name] = np.ascontiguousarray(np.asarray(w, np.float32).T).astype(
            np.float16
        )
    # Wo rows permuted to the head-slot order of attnT
    woT = np.ascontiguousarray(np.asarray(Wo, np.float32).T)
    perm = np.array(
        [64 * SLOT2HEAD[f // 64] + f % 64 for f in range(D)], np.int64
    )
    wts["woT"] = woT[perm].astype(np.float16)

    # band mask in [key, q] layout: valid iff q <= k <= q+15
    kk = np.arange(128)[:, None]
    qq = np.arange(QB)[None, :]
    band = ((kk >= qq) & (kk <= qq + WIN - 1)).astype(np.float16)
    ones = np.ones((128, 1), np.float16)

    in_maps = []
    for c in range(NCORES):
        bb, chunk = divmod(c, 4)
        g0 = chunk * CHUNK
        lo, hi = g0 - LP, g0 + CHUNK + RP
        xpad = np.zeros((TH, D), np.float32)
        src_lo, src_hi = max(lo, 0), min(hi, S)
        xpad[src_lo - lo: src_hi - lo] = x[bb, src_lo:src_hi]
        xT = np.ascontiguousarray(xpad.T).astype(np.float16)

        glob = g0 + np.arange(CHUNK)
        pos = glob[:, None] - LP + np.arange(WIN)[None, :]
        counts = ((pos < 0) | (pos >= S)).sum(axis=1).astype(np.float32)
        adj = np.zeros((QB, NBLK), np.float32)
        for b in range(NBLK):
            qn = QB if b < NBLK - 1 else QTAIL
            adj[:qn, b] = counts[QB * b: QB * b + qn]

        in_maps.append(
            {"xT": xT, "adj": adj, "bandT": band, "ones": ones, **wts}
        )
    return in_maps


def kernel(x, Wq, Wk, Wv, Wo):
    global LAST_RESULTS
    nc = _get_program()
    in_maps = _host_inputs(x, Wq, Wk, Wv, Wo)
    res = run_bass_kernel_spmd(
        nc, in_maps, core_ids=list(range(NCORES)), trace=TRACE
    )
    LAST_RESULTS = res
    out = np.empty((B, S, D), np.float32)
    for c in range(NCORES):
        bb, chunk = divmod(c, 4)
        out[bb, chunk * CHUNK:(chunk + 1) * CHUNK] = (
            res.results[c]["out"].astype(np.float32)
        )
    return out
